# revision 10
# baseline (speedup 1.0000x reference)
"""AdapterLayer (LN -> down-proj -> ReLU -> up-proj -> residual) on 8 TRN2 NeuronCores.

Sharding: pure data-parallel over the 16384 tokens (2048 tokens/core); adapter
params are replicated (tiny). No collectives.

v3 design: fp8e4 DoubleRow matmuls + LN folded into the contraction; no
on-device transposes except the small rd one; engine work spread so no single
vector engine exceeds the PE / DMA pace.

Host prep per core:
  xb   [TOK,D]    bf16  row-major x (for LN stats + residual)
  xT8  [P,KC,TOK] fp8   x pre-transposed to [d,tok] chunks (down-proj stationary)
  wd8  [P,KC,H]   fp8   256*(W_down*gamma)^T chunks   (down-proj moving)
  wu8  [P,HC,D]   fp8   256*W_up^T chunks             (up-proj moving)
  sconst [P,H]    bf16  rows 32g+0 = bd_eff, rows 32g+1 = 256*rowsum(W_down*gamma)
  bu256 [1,D]     bf16  256*b_up (ones-row seed into the up-proj PSUM)
The x256 weight scaling keeps fp8e4 out of its subnormal range (W ~ 0.02).

Device per 128-token tile (engine assignment):
  GpSimd: xb load + out store (SWDGE issue only).
  ScalarE: Sx = sum(x) and Sx2 = sum(x^2) via activation accum_out (Identity /
      Square passes over xb, dummy bf16 out); std256 = sqrt(32*v2048+65536*eps);
      rdT bf16 -> fp8 cast.
  DVE: v2048 = 2048*var = Sx2 - Sx^2/2048 (tiny); istdn = 1/std256;
      (std256, -mu) bf16 cols -> 32x32 block-transpose -> seed rows;
      rd = max(istdn*pd, 0) [tensor_scalar]; final combine
      out = pu*(1/256) + x [scalar_tensor_tensor].
  PE: down-proj = 8 DoubleRow MMs (xT8 stationary, wd8 moving) + 4 col-tiled
      K=2 seed MMs adding std256*bd_eff - mu*256*S per 32-token group
      (PSUM = 256*std*a_true); up-proj = 8 DoubleRow MMs (rdT8 stationary,
      c-outer for reuse) + ones-row b_up seeds (PSUM = 256*(U + b_up)).
  Sync: rd DMA-xbar-transpose only (transpose-pure queue).
"""

import numpy as np
import ml_dtypes

import concourse.bass as bass
import concourse.tile as tile
from concourse import mybir

from concourse.bass_utils import run_bass_kernel_spmd

# ---------------------------------------------------------------------------
# Workaround: the pinned walrus rejects >2 sem-waits on one instruction, but
# Tile's tail drain aggregates a wait per outstanding semaphore. Split them
# into one-wait-per-nop on the sync engine ahead of the drain.
from concourse.tile_sem_assignment import N_PROCS
from bass_rust import VectorClock, ScopedClock


def _drain_and_barrier_split(self, tick_clock, wait_clock):
    gc = tick_clock.global_clock
    for p in range(N_PROCS):
        if gc[p] == 0:
            continue
        c = [0] * N_PROCS
        c[p] = gc[p]
        nop = self.nc.sync.nop(nofuse=True, hint=f"drain_wait_p{p}")
        wait_clock.add_sem_waits(nop.ins, ScopedClock({None: VectorClock(c)}))
    self.nc.sync.drain()
    self.nc.all_engine_barrier()
    assert self.sems is not None
    popped = self.nc._tile_sem_poison_stack.pop()
    assert popped is self._sem_poison
    self.nc.clear_and_free_semaphores(list(self.sems.allocated().values()))
    self.nc.all_engine_barrier()


tile.TileContext._drain_and_barrier = _drain_and_barrier_split

# Same walrus limitation mid-kernel: any scheduled instruction may carry at
# most 2 sem-waits. Split excess waits onto same-engine NoOps committed just
# ahead of the instruction.
import bass_rust as _bass_rust

_MAX_WAITS = 1
_orig_commit = tile.TileContext._commit_instruction
_wsplit_counter = [0]


def _commit_instruction_split(self, inst, lazy_reg_writes=True):
    si = inst.sync_info
    if (
        si is not None
        and si.on_wait
        and len(si.on_wait) > _MAX_WAITS
        and inst.engine != mybir.EngineType.Unassigned
    ):
        waits = list(si.on_wait)
        keep = waits[-_MAX_WAITS:]
        extra = waits[:-_MAX_WAITS]
        for i in range(0, len(extra), _MAX_WAITS):
            _wsplit_counter[0] += 1
            nop = _bass_rust.InstNoOp(
                name=f"wsplit-{_wsplit_counter[0]}", ins=[], outs=[]
            )
            nop.engine = inst.engine
            nop.sync_info = _bass_rust.SyncInfo(
                on_wait=extra[i:i + _MAX_WAITS], on_update=[]
            )
            self._add_instruction(nop)
        inst.sync_info = _bass_rust.SyncInfo(
            on_wait=keep, on_update=list(si.on_update)
        )
    return _orig_commit(self, inst, lazy_reg_writes)


tile.TileContext._commit_instruction = _commit_instruction_split
# ---------------------------------------------------------------------------

B, S, D, H = 4, 4096, 2048, 512
EPS = 1e-5
NCORES = 8
TOK = B * S // NCORES  # tokens per core
P = 128
NT = TOK // P          # 16 token tiles per core
KC = D // P            # 16 contraction chunks for down-proj
HC = H // P            # 4 contraction chunks for up-proj
WSCALE = 256.0         # fp8 weight pre-scale (keeps W out of e4m3 subnormals)

F32 = mybir.dt.float32
BF16 = mybir.dt.bfloat16
F8 = mybir.dt.float8e4
DR = mybir.MatmulPerfMode.DoubleRow
AF = mybir.ActivationFunctionType
ALU = mybir.AluOpType


def build_nc():
    nc = bass.Bass("TRN2", target_bir_lowering=False, debug=False, num_devices=NCORES)

    xb_ext = nc.declare_dram_parameter("xb", [TOK, D], BF16, isOutput=False)
    xT8_ext = nc.declare_dram_parameter("xT8", [P, KC, TOK], F8, isOutput=False)
    wd8_ext = nc.declare_dram_parameter("wd8", [P, KC, H], F8, isOutput=False)
    wu8_ext = nc.declare_dram_parameter("wu8", [P, HC, D], F8, isOutput=False)
    sconst_ext = nc.declare_dram_parameter("sconst", [P, H], BF16, isOutput=False)
    bu256_ext = nc.declare_dram_parameter("bu256", [1, D], BF16, isOutput=False)
    out_ext = nc.declare_dram_parameter("out", [TOK, D], BF16, isOutput=True)

    with tile.TileContext(nc) as tc:
        with (
            tc.tile_pool(name="singles", bufs=1) as singles,
            tc.tile_pool(name="xbp", bufs=5) as xbp,
            tc.tile_pool(name="scrp", bufs=1) as scrp,
            tc.tile_pool(name="statp", bufs=6) as statp,
            tc.tile_pool(name="rp", bufs=3) as rp,
            tc.tile_pool(name="rtp", bufs=3) as rtp,
            tc.tile_pool(name="rt8p", bufs=3) as rt8p,
            tc.tile_pool(name="op", bufs=3) as op,
            tc.tile_pool(name="pdp", bufs=2, space="PSUM") as pdp,
            tc.tile_pool(name="pup", bufs=3, space="PSUM") as pup,
        ):
            # -------- persistent tiles --------
            wd8 = singles.tile([P, KC, H], F8)
            nc.gpsimd.dma_start(wd8[:], wd8_ext[:])
            wu8 = singles.tile([P, HC, D], F8)
            nc.gpsimd.dma_start(wu8[:], wu8_ext[:])
            sconst = singles.tile([P, H], BF16)
            nc.gpsimd.dma_start(sconst[:], sconst_ext[:])
            bu256 = singles.tile([1, D], BF16)
            nc.gpsimd.dma_start(bu256[:], bu256_ext[:])
            ones_row = singles.tile([1, P], BF16)
            nc.vector.memset(ones_row[:], 1.0)
            epst = singles.tile([P, 1], F32)
            nc.vector.memset(epst[:], 65536.0 * EPS)
            # resident pre-transposed fp8 x, loaded in 4 chunk-slices
            xT8 = singles.tile([P, KC, TOK], F8)
            for c in range(4):
                nc.scalar.dma_start(xT8[:, 4 * c:4 * (c + 1), :],
                                    xT8_ext[:, 4 * c:4 * (c + 1), :])

            def phase_a(t):
                """Load xb; LN stats -> seed rows (std256,-mu) + istdn."""
                x_sb = xbp.tile([P, D], BF16)
                nc.gpsimd.dma_start(x_sb[:], xb_ext[t * P:(t + 1) * P, :])

                # Sx on DVE (4x-mode single-src pass), Sx2 on ScalarE Square;
                # both use accum_out = per-partition row sum, dummy big out
                scr_d = scrp.tile([P, D], BF16)
                sx = statp.tile([P, 1], F32)
                nc.vector.tensor_scalar(scr_d[:], x_sb[:], 1.0, 0.0,
                                        ALU.mult, ALU.add, accum_out=sx[:])
                scr_s = scrp.tile([P, D], BF16)
                sx2 = statp.tile([P, 1], F32)
                nc.scalar.activation(scr_s[:], x_sb[:], AF.Square,
                                     accum_out=sx2[:])

                # v2048 = 2048*var = Sx2 - Sx^2/2048  (DVE tiny ops)
                sxsx = statp.tile([P, 1], F32)
                nc.vector.tensor_scalar(sxsx[:], sx[:], sx[:], None, ALU.mult)
                v2048 = statp.tile([P, 1], F32)
                nc.vector.scalar_tensor_tensor(
                    v2048[:], sxsx[:], -1.0 / 2048.0, sx2[:],
                    ALU.mult, ALU.add,
                )
                # std256 = sqrt(65536*var + 65536*eps) = 256*std
                std256 = statp.tile([P, 1], F32)
                nc.scalar.activation(std256[:], v2048[:], AF.Sqrt,
                                     bias=epst[:], scale=32.0)
                istdn = statp.tile([P, 1], F32)
                nc.vector.reciprocal(istdn[:], std256[:])

                # pack [std256, -mu] as bf16 cols, block-transpose to rows:
                # statsT[32g+0, j] = std256 of token 32g+j, [32g+1, j] = -mu
                spack = statp.tile([P, 32], BF16)
                nc.vector.memset(spack[:], 0.0)
                nc.vector.tensor_copy(spack[:, 0:1], std256[:])
                nc.vector.tensor_scalar_mul(spack[:, 1:2], sx[:], -1.0 / 2048.0)
                statsT = statp.tile([P, 32], BF16)
                nc.vector.transpose(statsT[:], spack[:])
                return x_sb, statsT, istdn

            def phase_b(t, x_sb, statsT, istdn):
                """Down-proj (fp8 DoubleRow) + LN seeds, ReLU, transpose."""
                pd = pdp.tile([P, H], F32)
                for k in range(KC // 2):
                    nc.tensor.matmul(
                        pd[:],
                        xT8[:, 2 * k:2 * k + 2, t * P:(t + 1) * P],
                        wd8[:, 2 * k:2 * k + 2, :],
                        start=(k == 0), stop=False, perf_mode=DR,
                    )
                # per-token-group rank-2 LN correction:
                # += std256*bd_eff[h] - mu*256*S[h]  (concurrent col-tiled MMs)
                for g in range(4):
                    nc.tensor.matmul(
                        pd[32 * g:32 * g + 32, :],
                        statsT[32 * g:32 * g + 2, 0:32],
                        sconst[32 * g:32 * g + 2, :],
                        start=False, stop=(g == 3),
                        tile_position=(32 * g, 32 * g),
                    )

                # rd = ReLU(istdn*pd) : exact adapter hidden state (ScalarE)
                rd = rp.tile([P, H], BF16)
                nc.scalar.activation(rd[:], pd[:], AF.Relu, scale=istdn[:])
                rdT = rtp.tile([P, HC, P], BF16)
                nc.sync.dma_start_transpose(rdT[:], rd[:])
                rdT8 = rt8p.tile([P, HC, P], F8)
                nc.gpsimd.tensor_copy(rdT8[:], rdT[:])
                return x_sb, rdT8

            def phase_c(t, x_sb, rdT8):
                """Up-proj (fp8 DoubleRow) + b_up seed + combine + store."""
                o_sb = op.tile([P, D], BF16)
                for q2 in range(2):
                    pu = pup.tile([P, 1024], F32)
                    for c in range(2):  # k-pair outer: stationary reused 2x
                        for q in range(2):
                            n0 = (q2 * 2 + q) * 512
                            nc.tensor.matmul(
                                pu[:, q * 512:(q + 1) * 512],
                                rdT8[:, 2 * c:2 * c + 2, :],
                                wu8[:, 2 * c:2 * c + 2, n0:n0 + 512],
                                start=(c == 0), stop=False, perf_mode=DR,
                            )
                    for q in range(2):  # += 256*b_up (ones-row seed)
                        n0 = (q2 * 2 + q) * 512
                        nc.tensor.matmul(
                            pu[:, q * 512:(q + 1) * 512],
                            ones_row[:],
                            bu256[:, n0:n0 + 512],
                            start=False, stop=True,
                        )
                    sl = slice(q2 * 1024, (q2 + 1) * 1024)
                    nc.vector.scalar_tensor_tensor(
                        o_sb[:, sl], pu[:], 1.0 / WSCALE, x_sb[:, sl],
                        ALU.mult, ALU.add,
                    )
                nc.gpsimd.dma_start(out_ext[t * P:(t + 1) * P, :], o_sb[:])

            # staggered software pipeline; emit consumer phases FIRST each
            # iteration so per-engine FIFOs don't head-of-line block the
            # current tile's critical ops behind future tiles' prefetch work
            h1, h2_ = {}, {}
            for t in range(NT + 3):
                if 3 <= t:
                    phase_c(t - 3, *h2_.pop(t - 3))
                if 2 <= t < NT + 2:
                    h2_[t - 2] = phase_b(t - 2, *h1.pop(t - 2))
                if t < NT:
                    h1[t] = phase_a(t)

    return nc


_NC_CACHE = None


def _get_nc():
    global _NC_CACHE
    if _NC_CACHE is None:
        _NC_CACHE = build_nc()
    return _NC_CACHE


def make_in_maps(x, ln_gamma, ln_beta, W_down, b_down, W_up, b_up):
    bf = ml_dtypes.bfloat16
    f8 = ml_dtypes.float8_e4m3

    x2d = np.asarray(x, dtype=np.float32).reshape(B * S, D)
    xb_all = np.ascontiguousarray(x2d).astype(bf)

    # Fold LN affine (gamma/beta) into the down projection exactly:
    #   W_down @ (yhat*gamma + beta) = (W_down*gamma) @ yhat + W_down @ beta
    Wd = np.asarray(W_down, dtype=np.float64)
    gamma = np.asarray(ln_gamma, dtype=np.float64)
    beta = np.asarray(ln_beta, dtype=np.float64)
    wd_eff = Wd * gamma[None, :]          # [H, D]
    bd_eff = np.asarray(b_down, dtype=np.float64) + Wd @ beta  # [H]
    srow = WSCALE * wd_eff.sum(axis=1)    # [H] = 256*S

    wd8_host = np.ascontiguousarray(
        (WSCALE * wd_eff).T.reshape(KC, P, H).transpose(1, 0, 2)
    ).astype(f8)
    wu8_host = np.ascontiguousarray(
        (WSCALE * np.asarray(W_up, dtype=np.float64)).T
        .reshape(HC, P, D).transpose(1, 0, 2)
    ).astype(f8)

    # seed-MM moving rows: partition 32g+0 = bd_eff (pairs std256 row),
    # partition 32g+1 = 256*S (pairs -mu row)
    sconst_host = np.zeros((P, H), dtype=np.float64)
    for g in range(4):
        sconst_host[32 * g + 0, :] = bd_eff
        sconst_host[32 * g + 1, :] = srow
    sconst_host = sconst_host.astype(bf)

    bu256_host = (WSCALE * np.asarray(b_up, dtype=np.float64)).reshape(1, D).astype(bf)

    in_maps = []
    for i in range(NCORES):
        sl = x2d[i * TOK:(i + 1) * TOK]  # [TOK, D] f32
        xT8_host = np.ascontiguousarray(
            sl.T.reshape(KC, P, TOK).transpose(1, 0, 2)).astype(f8)
        in_maps.append({
            "xb": xb_all[i * TOK:(i + 1) * TOK],
            "xT8": xT8_host,
            "wd8": wd8_host,
            "wu8": wu8_host,
            "sconst": sconst_host,
            "bu256": bu256_host,
        })
    return in_maps


def gather_out(results):
    return np.concatenate(
        [np.asarray(results[i]["out"]).astype(np.float32) for i in range(NCORES)],
        axis=0,
    ).reshape(B, S, D)


def kernel(x, ln_gamma, ln_beta, W_down, b_down, W_up, b_up):
    nc = _get_nc()
    in_maps = make_in_maps(x, ln_gamma, ln_beta, W_down, b_down, W_up, b_up)
    res = run_bass_kernel_spmd(nc, in_maps, core_ids=list(range(NCORES)))
    return gather_out(res.results)


# revision 15
# speedup vs baseline: 1.0738x; 1.0738x over previous
"""AdapterLayer (LN -> down-proj -> ReLU -> up-proj -> residual) on 8 TRN2 NeuronCores.

Sharding: pure data-parallel over the 16384 tokens (2048 tokens/core); adapter
params are replicated (tiny). No collectives.

v3 design: fp8e4 DoubleRow matmuls + LN folded into the contraction; no
on-device transposes except the small rd one; engine work spread so no single
vector engine exceeds the PE / DMA pace.

Host prep per core:
  xb   [TOK,D]    bf16  row-major x (for LN stats + residual)
  xT8  [P,KC,TOK] fp8   x pre-transposed to [d,tok] chunks (down-proj stationary)
  wd8  [P,KC,H]   fp8   256*(W_down*gamma)^T chunks   (down-proj moving)
  wu8  [P,HC,D]   fp8   256*W_up^T chunks             (up-proj moving)
  sconst [P,H]    bf16  rows 32g+0 = bd_eff, rows 32g+1 = 256*rowsum(W_down*gamma)
  bu256 [1,D]     bf16  256*b_up (ones-row seed into the up-proj PSUM)
The x256 weight scaling keeps fp8e4 out of its subnormal range (W ~ 0.02).

Device per 128-token tile (engine assignment):
  GpSimd: xb load + out store (SWDGE issue only).
  ScalarE: Sx = sum(x) and Sx2 = sum(x^2) via activation accum_out (Identity /
      Square passes over xb, dummy bf16 out); std256 = sqrt(32*v2048+65536*eps);
      rdT bf16 -> fp8 cast.
  DVE: v2048 = 2048*var = Sx2 - Sx^2/2048 (tiny); istdn = 1/std256;
      (std256, -mu) bf16 cols -> 32x32 block-transpose -> seed rows;
      rd = max(istdn*pd, 0) [tensor_scalar]; final combine
      out = pu*(1/256) + x [scalar_tensor_tensor].
  PE: down-proj = 8 DoubleRow MMs (xT8 stationary, wd8 moving) + 4 col-tiled
      K=2 seed MMs adding std256*bd_eff - mu*256*S per 32-token group
      (PSUM = 256*std*a_true); up-proj = 8 DoubleRow MMs (rdT8 stationary,
      c-outer for reuse) + ones-row b_up seeds (PSUM = 256*(U + b_up)).
  Sync: rd DMA-xbar-transpose only (transpose-pure queue).
"""

import numpy as np
import ml_dtypes

import concourse.bass as bass
import concourse.tile as tile
from concourse import mybir

from concourse.bass_utils import run_bass_kernel_spmd

# ---------------------------------------------------------------------------
# Workaround: the pinned walrus rejects >2 sem-waits on one instruction, but
# Tile's tail drain aggregates a wait per outstanding semaphore. Split them
# into one-wait-per-nop on the sync engine ahead of the drain.
from concourse.tile_sem_assignment import N_PROCS
from bass_rust import VectorClock, ScopedClock


def _drain_and_barrier_split(self, tick_clock, wait_clock):
    gc = tick_clock.global_clock
    for p in range(N_PROCS):
        if gc[p] == 0:
            continue
        c = [0] * N_PROCS
        c[p] = gc[p]
        nop = self.nc.sync.nop(nofuse=True, hint=f"drain_wait_p{p}")
        wait_clock.add_sem_waits(nop.ins, ScopedClock({None: VectorClock(c)}))
    self.nc.sync.drain()
    self.nc.all_engine_barrier()
    assert self.sems is not None
    popped = self.nc._tile_sem_poison_stack.pop()
    assert popped is self._sem_poison
    self.nc.clear_and_free_semaphores(list(self.sems.allocated().values()))
    self.nc.all_engine_barrier()


tile.TileContext._drain_and_barrier = _drain_and_barrier_split

# Same walrus limitation mid-kernel: any scheduled instruction may carry at
# most 2 sem-waits. Split excess waits onto same-engine NoOps committed just
# ahead of the instruction.
import bass_rust as _bass_rust

_MAX_WAITS = 1
_orig_commit = tile.TileContext._commit_instruction
_wsplit_counter = [0]


def _commit_instruction_split(self, inst, lazy_reg_writes=True):
    si = inst.sync_info
    if (
        si is not None
        and si.on_wait
        and len(si.on_wait) > _MAX_WAITS
        and inst.engine != mybir.EngineType.Unassigned
    ):
        waits = list(si.on_wait)
        keep = waits[-_MAX_WAITS:]
        extra = waits[:-_MAX_WAITS]
        for i in range(0, len(extra), _MAX_WAITS):
            _wsplit_counter[0] += 1
            nop = _bass_rust.InstNoOp(
                name=f"wsplit-{_wsplit_counter[0]}", ins=[], outs=[]
            )
            nop.engine = inst.engine
            nop.sync_info = _bass_rust.SyncInfo(
                on_wait=extra[i:i + _MAX_WAITS], on_update=[]
            )
            self._add_instruction(nop)
        inst.sync_info = _bass_rust.SyncInfo(
            on_wait=keep, on_update=list(si.on_update)
        )
    return _orig_commit(self, inst, lazy_reg_writes)


tile.TileContext._commit_instruction = _commit_instruction_split
# ---------------------------------------------------------------------------

B, S, D, H = 4, 4096, 2048, 512
EPS = 1e-5
NCORES = 8
TOK = B * S // NCORES  # tokens per core
P = 128
NT = TOK // P          # 16 token tiles per core
KC = D // P            # 16 contraction chunks for down-proj
HC = H // P            # 4 contraction chunks for up-proj
WSCALE = 256.0         # fp8 weight pre-scale (keeps W out of e4m3 subnormals)

F32 = mybir.dt.float32
BF16 = mybir.dt.bfloat16
F8 = mybir.dt.float8e4
DR = mybir.MatmulPerfMode.DoubleRow
AF = mybir.ActivationFunctionType
ALU = mybir.AluOpType


def build_nc():
    nc = bass.Bass("TRN2", target_bir_lowering=False, debug=False, num_devices=NCORES)

    xb_ext = nc.declare_dram_parameter("xb", [TOK, D], BF16, isOutput=False)
    xT8_ext = nc.declare_dram_parameter("xT8", [P, KC, TOK], F8, isOutput=False)
    wd8_ext = nc.declare_dram_parameter("wd8", [P, KC, H], F8, isOutput=False)
    wu8_ext = nc.declare_dram_parameter("wu8", [P, HC, D], F8, isOutput=False)
    sconst_ext = nc.declare_dram_parameter("sconst", [P, H], BF16, isOutput=False)
    bu256_ext = nc.declare_dram_parameter("bu256", [1, D], BF16, isOutput=False)
    out_ext = nc.declare_dram_parameter("out", [TOK, D], BF16, isOutput=True)

    with tile.TileContext(nc) as tc:
        with (
            tc.tile_pool(name="singles", bufs=1) as singles,
            tc.tile_pool(name="xbp", bufs=6) as xbp,
            tc.tile_pool(name="scrp", bufs=1) as scrp,
            tc.tile_pool(name="statp", bufs=7) as statp,
            tc.tile_pool(name="rp", bufs=3) as rp,
            tc.tile_pool(name="rtp", bufs=4) as rtp,
            tc.tile_pool(name="rt8p", bufs=4) as rt8p,
            tc.tile_pool(name="op", bufs=3) as op,
            tc.tile_pool(name="pdp", bufs=2, space="PSUM") as pdp,
            tc.tile_pool(name="pup", bufs=3, space="PSUM") as pup,
        ):
            # -------- persistent tiles --------
            wd8 = singles.tile([P, KC, H], F8)
            nc.gpsimd.dma_start(wd8[:], wd8_ext[:])
            wu8 = singles.tile([P, HC, D], F8)
            nc.gpsimd.dma_start(wu8[:], wu8_ext[:])
            sconst = singles.tile([P, H], BF16)
            nc.gpsimd.dma_start(sconst[:], sconst_ext[:])
            bu256 = singles.tile([1, D], BF16)
            nc.gpsimd.dma_start(bu256[:], bu256_ext[:])
            ones_row = singles.tile([1, P], BF16)
            nc.vector.memset(ones_row[:], 1.0)
            epst = singles.tile([P, 1], F32)
            nc.vector.memset(epst[:], 65536.0 * EPS)
            # resident pre-transposed fp8 x, loaded in 4 chunk-slices
            xT8 = singles.tile([P, KC, TOK], F8)
            for c in range(4):
                nc.scalar.dma_start(xT8[:, 4 * c:4 * (c + 1), :],
                                    xT8_ext[:, 4 * c:4 * (c + 1), :])

            def phase_a(t):
                """Load xb; LN stats -> seed rows (std256,-mu) + istdn."""
                x_sb = xbp.tile([P, D], BF16)
                nc.gpsimd.dma_start(x_sb[:], xb_ext[t * P:(t + 1) * P, :])

                # Sx on DVE (tensor_scalar + accum reduce),
                # Sx2 on ScalarE Square; accum_out = per-partition row sum
                scr_d = scrp.tile([P, D], BF16)
                sx = statp.tile([P, 1], F32)
                nc.vector.tensor_scalar(scr_d[:], x_sb[:], 1.0, 0.0,
                                        ALU.mult, ALU.add, accum_out=sx[:])
                scr_s = scrp.tile([P, D], BF16)
                sx2 = statp.tile([P, 1], F32)
                nc.scalar.activation(scr_s[:], x_sb[:], AF.Square,
                                     accum_out=sx2[:])

                # v2048 = 2048*var = Sx2 - Sx^2/2048  (DVE tiny ops)
                sxsx = statp.tile([P, 1], F32)
                nc.vector.tensor_scalar(sxsx[:], sx[:], sx[:], None, ALU.mult)
                v2048 = statp.tile([P, 1], F32)
                nc.vector.scalar_tensor_tensor(
                    v2048[:], sxsx[:], -1.0 / 2048.0, sx2[:],
                    ALU.mult, ALU.add,
                )
                # std256 = sqrt(65536*var + 65536*eps) = 256*std
                std256 = statp.tile([P, 1], F32)
                nc.scalar.activation(std256[:], v2048[:], AF.Sqrt,
                                     bias=epst[:], scale=32.0)
                istdn = statp.tile([P, 1], F32)
                nc.vector.reciprocal(istdn[:], std256[:])

                # pack [std256, -mu] as bf16 cols, block-transpose to rows:
                # statsT[32g+0, j] = std256 of token 32g+j, [32g+1, j] = -mu
                spack = statp.tile([P, 32], BF16)
                nc.vector.memset(spack[:], 0.0)
                nc.vector.tensor_copy(spack[:, 0:1], std256[:])
                nc.vector.tensor_scalar_mul(spack[:, 1:2], sx[:], -1.0 / 2048.0)
                statsT = statp.tile([P, 32], BF16)
                nc.vector.transpose(statsT[:], spack[:])
                return x_sb, statsT, istdn

            def phase_b(t, x_sb, statsT, istdn):
                """Down-proj (fp8 DoubleRow) + LN seeds, ReLU, transpose."""
                pd = pdp.tile([P, H], F32)
                for k in range(KC // 2):
                    nc.tensor.matmul(
                        pd[:],
                        xT8[:, 2 * k:2 * k + 2, t * P:(t + 1) * P],
                        wd8[:, 2 * k:2 * k + 2, :],
                        start=(k == 0), stop=False, perf_mode=DR,
                    )
                # per-token-group rank-2 LN correction:
                # += std256*bd_eff[h] - mu*256*S[h]  (concurrent col-tiled MMs)
                for g in range(4):
                    nc.tensor.matmul(
                        pd[32 * g:32 * g + 32, :],
                        statsT[32 * g:32 * g + 2, 0:32],
                        sconst[32 * g:32 * g + 2, :],
                        start=False, stop=(g == 3),
                        tile_position=(32 * g, 32 * g),
                    )

                # rd = ReLU(istdn*pd) : exact adapter hidden state (ScalarE)
                rd = rp.tile([P, H], BF16)
                nc.scalar.activation(rd[:], pd[:], AF.Relu, scale=istdn[:])
                rdT = rtp.tile([P, HC, P], BF16)
                nc.sync.dma_start_transpose(rdT[:], rd[:])
                return x_sb, rdT

            def phase_bc(t, x_sb, rdT):
                """fp8 cast of the transposed hidden (own phase: keeps the
                ScalarE FIFO free of ops whose deps are still in flight)."""
                rdT8 = rt8p.tile([P, HC, P], F8)
                nc.scalar.copy(rdT8[:], rdT[:])
                return x_sb, rdT8

            def phase_c(t, x_sb, rdT8):
                """Up-proj (fp8 DoubleRow) + b_up seed + combine + store."""
                o_sb = op.tile([P, D], BF16)
                for q2 in range(2):
                    pu = pup.tile([P, 1024], F32)
                    for c in range(2):  # k-pair outer: stationary reused 2x
                        for q in range(2):
                            n0 = (q2 * 2 + q) * 512
                            nc.tensor.matmul(
                                pu[:, q * 512:(q + 1) * 512],
                                rdT8[:, 2 * c:2 * c + 2, :],
                                wu8[:, 2 * c:2 * c + 2, n0:n0 + 512],
                                start=(c == 0), stop=False, perf_mode=DR,
                            )
                    for q in range(2):  # += 256*b_up (ones-row seed)
                        n0 = (q2 * 2 + q) * 512
                        nc.tensor.matmul(
                            pu[:, q * 512:(q + 1) * 512],
                            ones_row[:],
                            bu256[:, n0:n0 + 512],
                            start=False, stop=True,
                        )
                    sl = slice(q2 * 1024, (q2 + 1) * 1024)
                    nc.vector.scalar_tensor_tensor(
                        o_sb[:, sl], pu[:], 1.0 / WSCALE, x_sb[:, sl],
                        ALU.mult, ALU.add,
                    )
                nc.gpsimd.dma_start(out_ext[t * P:(t + 1) * P, :], o_sb[:])

            # staggered software pipeline: A@t, B@t+2, cast@t+3, C@t+4.
            # Emit consumer phases FIRST each iteration so per-engine FIFOs
            # don't head-of-line block the current tile's critical ops
            # behind future tiles' prefetch work.
            h1, h2_, h3 = {}, {}, {}
            for t in range(NT + 4):
                if 4 <= t:
                    phase_c(t - 4, *h3.pop(t - 4))
                if 3 <= t < NT + 3:
                    h3[t - 3] = phase_bc(t - 3, *h2_.pop(t - 3))
                if 2 <= t < NT + 2:
                    h2_[t - 2] = phase_b(t - 2, *h1.pop(t - 2))
                if t < NT:
                    h1[t] = phase_a(t)

    return nc


_NC_CACHE = None


def _get_nc():
    global _NC_CACHE
    if _NC_CACHE is None:
        _NC_CACHE = build_nc()
    return _NC_CACHE


def make_in_maps(x, ln_gamma, ln_beta, W_down, b_down, W_up, b_up):
    bf = ml_dtypes.bfloat16
    f8 = ml_dtypes.float8_e4m3

    x2d = np.asarray(x, dtype=np.float32).reshape(B * S, D)
    xb_all = np.ascontiguousarray(x2d).astype(bf)

    # Fold LN affine (gamma/beta) into the down projection exactly:
    #   W_down @ (yhat*gamma + beta) = (W_down*gamma) @ yhat + W_down @ beta
    Wd = np.asarray(W_down, dtype=np.float64)
    gamma = np.asarray(ln_gamma, dtype=np.float64)
    beta = np.asarray(ln_beta, dtype=np.float64)
    wd_eff = Wd * gamma[None, :]          # [H, D]
    bd_eff = np.asarray(b_down, dtype=np.float64) + Wd @ beta  # [H]
    srow = WSCALE * wd_eff.sum(axis=1)    # [H] = 256*S

    wd8_host = np.ascontiguousarray(
        (WSCALE * wd_eff).T.reshape(KC, P, H).transpose(1, 0, 2)
    ).astype(f8)
    wu8_host = np.ascontiguousarray(
        (WSCALE * np.asarray(W_up, dtype=np.float64)).T
        .reshape(HC, P, D).transpose(1, 0, 2)
    ).astype(f8)

    # seed-MM moving rows: partition 32g+0 = bd_eff (pairs std256 row),
    # partition 32g+1 = 256*S (pairs -mu row)
    sconst_host = np.zeros((P, H), dtype=np.float64)
    for g in range(4):
        sconst_host[32 * g + 0, :] = bd_eff
        sconst_host[32 * g + 1, :] = srow
    sconst_host = sconst_host.astype(bf)

    bu256_host = (WSCALE * np.asarray(b_up, dtype=np.float64)).reshape(1, D).astype(bf)

    in_maps = []
    for i in range(NCORES):
        sl = x2d[i * TOK:(i + 1) * TOK]  # [TOK, D] f32
        xT8_host = np.ascontiguousarray(
            sl.T.reshape(KC, P, TOK).transpose(1, 0, 2)).astype(f8)
        in_maps.append({
            "xb": xb_all[i * TOK:(i + 1) * TOK],
            "xT8": xT8_host,
            "wd8": wd8_host,
            "wu8": wu8_host,
            "sconst": sconst_host,
            "bu256": bu256_host,
        })
    return in_maps


def gather_out(results):
    return np.concatenate(
        [np.asarray(results[i]["out"]).astype(np.float32) for i in range(NCORES)],
        axis=0,
    ).reshape(B, S, D)


def kernel(x, ln_gamma, ln_beta, W_down, b_down, W_up, b_up):
    nc = _get_nc()
    in_maps = make_in_maps(x, ln_gamma, ln_beta, W_down, b_down, W_up, b_up)
    res = run_bass_kernel_spmd(nc, in_maps, core_ids=list(range(NCORES)))
    return gather_out(res.results)
